# revision 34
# baseline (speedup 1.0000x reference)
"""Trainium2 Bass kernel for nn_Encoder (FC+BN+LeakyReLU -> 2-layer LSTM -> last h).

Data-parallel over 8 NeuronCores: each core handles B_SHARD=256 of the 2048
batch rows; small FC/BN/LSTM params are replicated.

Per-core pipeline (all shapes [partition, free]):
  Stage A (FC):  x[b] is (180,120) feature-major; matmul with K split 128+52
                 accumulates fc_w'@x into PSUM, ACT adds bias (BN folded on
                 host), DVE applies LeakyReLU via max(0.01*z, z).
                 Produces h_fc (128, 256*120) fp16 resident in SBUF.
  Stage B (LSTM): 2 layers, 120 steps, software-pipelined (layer 1 lags
                 LAG steps so its work can fill layer 0's chain bubbles).
                 Per step+layer, gates in order (i|f|o|g) live in one
                 (128, 1024) fp32 PSUM tile (2 banks):
                   - i/f/o biases injected via two small matmuls against a
                     constant 0/1 rhs (off the critical path); g's bias rides
                     the Tanh activation's per-partition bias operand
                   - 4 x-part matmuls (K=128, N=256) from h_fc / h0
                   - 4 h-part matmuls from the recurrent state
                 Elementwise chain per step (sigmoid/tanh in one ACT table
                 set; all DVE/Pool ops are plain fp16 tensor_tensor → 2x):
                   sifo = sigmoid(psum[:, :768])             (ACT)
                   tg   = tanh(psum[:, 768:] + b_g)          (ACT)
                   t1   = tg * si                            (DVE TT)  = i*g
                   t2   = sf * c                             (Pool TT)
                   c    = t1 + t2                            (DVE TT)
                   th   = tanh(c)                            (ACT)
                   h    = th * so                            (DVE TT)
  Output: h1 at t=119 DMA'd out as (128, 256) fp16 per core; host
          transposes/concats/casts to (2048, 128) fp32.
"""

from contextlib import ExitStack

import numpy as np

import concourse.bass as bass
import concourse.mybir as mybir
import concourse.tile as tile
from concourse import bacc
from concourse.bass_utils import run_bass_kernel_spmd

H = 128
T = 120
F_IN = 180
N_CORES = 8
B_SHARD = 256
BN_EPS = 1e-5
SLOPE = 0.01

FP32 = mybir.dt.float32
FP16 = mybir.dt.float16
AF = mybir.ActivationFunctionType
ALU = mybir.AluOpType

FC_NB = 16          # batches per x DMA group
FC_MM_NB = 4        # batches per FC matmul (N = 4*120 = 480)


def build_kernel():
    nc = bacc.Bacc("TRN2", target_bir_lowering=False, debug=False,
                   num_devices=N_CORES)

    xa_d = nc.dram_tensor("xa", (128, B_SHARD * T), FP16, kind="ExternalInput")
    xb_d = nc.dram_tensor("xb", (B_SHARD // (2 * FC_NB), 64 + F_IN - 128, FC_NB * T),
                          FP16, kind="ExternalInput")
    fcw_d = nc.dram_tensor("fcw_t", (F_IN, H), FP16, kind="ExternalInput")
    fcb_d = nc.dram_tensor("fcb", (H, 1), FP32, kind="ExternalInput")
    wih_d = [nc.dram_tensor(f"wih{l}_t", (H, 4 * H), FP16, kind="ExternalInput")
             for l in range(2)]
    whh_d = [nc.dram_tensor(f"whh{l}_t", (H, 4 * H), FP16, kind="ExternalInput")
             for l in range(2)]
    bias_d = nc.dram_tensor("bias_pack", (2, 512), FP16, kind="ExternalInput")
    ones_d = nc.dram_tensor("ones_pack", (2, 512), FP16, kind="ExternalInput")

    out_d = nc.dram_tensor("out", (H, B_SHARD), FP16, kind="ExternalOutput")

    ctx = ExitStack()
    with ctx:
        tc = ctx.enter_context(tile.TileContext(nc))
        consts = ctx.enter_context(tc.tile_pool(name="consts", bufs=1))
        state = ctx.enter_context(tc.tile_pool(name="state", bufs=1))

        # ---- constants into SBUF ----
        fcw_a = consts.tile([128, H], FP16, tag="fcw_a")
        # two copies of the 52-row tail weight, at base partitions 0 and 64,
        # matching the packed xb layout (matmul needs equal base partitions)
        fcw_b2 = consts.tile([64 + F_IN - 128, H], FP16, tag="fcw_b")
        nc.sync.dma_start(out=fcw_a, in_=fcw_d.ap()[0:128, :])
        nc.sync.dma_start(out=fcw_b2[0:F_IN - 128], in_=fcw_d.ap()[128:F_IN, :])
        nc.sync.dma_start(out=fcw_b2[64:64 + F_IN - 128], in_=fcw_d.ap()[128:F_IN, :])
        fcb = consts.tile([H, 1], FP32, tag="fcb")
        nc.sync.dma_start(out=fcb, in_=fcb_d.ap())
        wihT = []
        whhT = []
        for l in range(2):
            wt = consts.tile([H, 4 * H], FP16, tag=f"wih{l}")
            nc.sync.dma_start(out=wt, in_=wih_d[l].ap())
            wihT.append(wt)
            ht = consts.tile([H, 4 * H], FP16, tag=f"whh{l}")
            nc.sync.dma_start(out=ht, in_=whh_d[l].ap())
            whhT.append(ht)
        bias_t = consts.tile([2, 512], FP16, tag="bias")
        nc.sync.dma_start(out=bias_t, in_=bias_d.ap())
        ones_t = consts.tile([2, 512], FP16, tag="ones")
        nc.sync.dma_start(out=ones_t, in_=ones_d.ap())


        # ---- persistent state ----
        h_fc = state.tile([128, B_SHARD, T], FP16, tag="h_fc")
        h0_all = state.tile([128, T + 1, B_SHARD], FP16, tag="h0_all")
        h1_ring = state.tile([128, 2, B_SHARD], FP16, tag="h1_ring")
        c2 = [state.tile([128, B_SHARD], FP16, tag=f"c2_{l}", name=f"c2_{l}")
              for l in range(2)]
        nc.vector.memset(h0_all[:, 0, :], 0.0)
        nc.vector.memset(h1_ring[:, 0, :], 0.0)
        nc.vector.memset(c2[0], 0.0)
        nc.vector.memset(c2[1], 0.0)

        # ---- Stage A: FC + bias + LeakyReLU ----
        with tc.tile_pool(name="fc_x", bufs=3) as xpool, \
             tc.tile_pool(name="fc_ps", bufs=4, space="PSUM") as fc_psum, \
             tc.tile_pool(name="fc_z", bufs=4) as zpool:
            FB = F_IN - 128  # 52 tail rows; two groups' tails share one DMA
            for grp in range(B_SHARD // FC_NB):
                b0 = grp * FC_NB
                xa = xpool.tile([128, FC_NB * T], FP16, tag="xa")
                nc.sync.dma_start(
                    out=xa, in_=xa_d.ap()[:, b0 * T:(b0 + FC_NB) * T])
                if grp % 2 == 0:
                    xb = xpool.tile([64 + FB, FC_NB * T], FP16, tag="xb", name="xb")
                    nc.sync.dma_start(out=xb, in_=xb_d.ap()[grp // 2])
                    xb_lo, fcw_b = xb[0:FB], fcw_b2[0:FB]
                else:
                    xb_lo, fcw_b = xb[64:64 + FB], fcw_b2[64:64 + FB]
                for mm in range(FC_NB // FC_MM_NB):
                    s = mm * FC_MM_NB
                    ps = fc_psum.tile([128, FC_MM_NB * T], FP32, tag="ps")
                    rhs_a = xa[:, s * T:(s + FC_MM_NB) * T]
                    rhs_b = xb_lo[:, s * T:(s + FC_MM_NB) * T]
                    nc.tensor.matmul(ps, fcw_a, rhs_a, start=True, stop=False)
                    nc.tensor.matmul(ps, fcw_b, rhs_b, start=False, stop=True)
                    z = zpool.tile([128, FC_MM_NB * T], FP16, tag="z")
                    nc.scalar.activation(out=z, in_=ps, func=AF.Identity,
                                         bias=fcb, scale=1.0)
                    dst = h_fc[:, b0 + s:b0 + s + FC_MM_NB, :].rearrange(
                        "f b t -> f (b t)")
                    nc.vector.scalar_tensor_tensor(
                        out=dst, in0=z, scalar=SLOPE, in1=z,
                        op0=ALU.mult, op1=ALU.max)

        # ---- Stage B: LSTM ----
        hfc_t = h_fc  # [128, b, t]; x-part rhs for layer 0 at step t: h_fc[:, :, t]
        LAG = 3       # layer-1 pipeline lag (ticks) so it fills layer-0 bubbles
        with tc.tile_pool(name="g_ps", bufs=2, space="PSUM") as gpsum, \
             tc.tile_pool(name="ifgo", bufs=2) as gpool, \
             tc.tile_pool(name="sc", bufs=3) as scpool:

            def lstm_step_a(l, t):
                """Matmuls + sigmoids + cell update; returns (th-input deps)."""
                if l == 0:
                    rhs_x = hfc_t[:, :, t]
                    rhs_h = h0_all[:, t, :]
                else:
                    rhs_x = h0_all[:, t + 1, :]
                    rhs_h = h1_ring[:, t % 2, :]
                ps = gpsum.tile([128, 1024], FP32, tag=f"g{l}", name=f"ps{l}")
                # biases via matmuls against 0/1 rhs: i/f (bank0, K=2, N=512),
                # g doubled + o (bank1, K=2, N=512)
                nc.tensor.matmul(ps[:, 0:512], bias_t[:, l * 256:l * 256 + 128],
                                 ones_t, start=True, stop=False)
                nc.tensor.matmul(ps[:, 512:1024],
                                 bias_t[:, l * 256 + 128:l * 256 + 256],
                                 ones_t, start=True, stop=False)
                for g in range(4):
                    nc.tensor.matmul(ps[:, g * 256:(g + 1) * 256],
                                     wihT[l][:, g * 128:(g + 1) * 128], rhs_x,
                                     start=False, stop=False)
                for g in range(4):
                    nc.tensor.matmul(ps[:, g * 256:(g + 1) * 256],
                                     whhT[l][:, g * 128:(g + 1) * 128], rhs_h,
                                     start=False, stop=(g >= 1))
                # one sigmoid over i, f, g', o (g rows pre-doubled:
                # tanh(z) = 2*sigmoid(2z) - 1)
                sifg = gpool.tile([128, 1024], FP16, tag=f"sifg{l}", name=f"sifg{l}")
                nc.scalar.activation(out=sifg[:, 0:768], in_=ps[:, 0:768],
                                     func=AF.Sigmoid)
                nc.scalar.activation(out=sifg[:, 768:1024], in_=ps[:, 768:1024],
                                     func=AF.Sigmoid)
                si = sifg[:, 0:256]
                sf = sifg[:, 256:512]
                sg = sifg[:, 512:768]
                so = sifg[:, 768:1024]
                t2 = scpool.tile([128, 256], FP16, tag=f"t2_{l}", name=f"t2_{l}")
                nc.vector.tensor_mul(t2, sf, c2[l])
                t1 = scpool.tile([128, 256], FP16, tag=f"t1_{l}", name=f"t1_{l}")
                nc.vector.scalar_tensor_tensor(out=t1, in0=sg, scalar=-0.5,
                                               in1=si, op0=ALU.add, op1=ALU.mult)
                nc.vector.scalar_tensor_tensor(out=c2[l], in0=t1, scalar=2.0,
                                               in1=t2, op0=ALU.mult, op1=ALU.add)
                return so

            def lstm_step_b(l, t, so):
                h_dst = (h0_all[:, t + 1, :] if l == 0
                         else h1_ring[:, (t + 1) % 2, :])
                th = scpool.tile([128, 256], FP16, tag=f"th{l}", name=f"th{l}")
                nc.scalar.activation(out=th, in_=c2[l], func=AF.Tanh)
                nc.vector.tensor_mul(h_dst, th, so)

            for tick in range(T + LAG):
                so1 = lstm_step_a(1, tick - LAG) if tick >= LAG else None
                so0 = lstm_step_a(0, tick) if tick < T else None
                if so1 is not None:
                    lstm_step_b(1, tick - LAG, so1)
                if so0 is not None:
                    lstm_step_b(0, tick, so0)

            nc.sync.dma_start(out=out_d.ap(), in_=h1_ring[:, T % 2, :])

    nc.compile()
    return nc


def prep_inputs(inputs):
    """Host-side weight preprocessing; returns per-core in_maps."""
    inp = {k: np.asarray(v, dtype=np.float32) for k, v in inputs.items()}
    s = 1.0 / np.sqrt(inp["bn_var"] + BN_EPS)
    scale_p = s * inp["bn_g"]
    fcw = scale_p[:, None] * inp["fc_w"]          # (128, 180)
    fcb = (inp["fc_b"] - inp["bn_mean"]) * scale_p + inp["bn_b"]

    # Gate order on-chip matches PyTorch rows: i, f, g, o.  The g rows are
    # doubled so tanh(z) = 2*sigmoid(2z) - 1 comes out of the sigmoid op.
    R = np.ones((4 * H, 1), np.float32)
    R[2 * H:3 * H] = 2.0

    def lstm_w(wih, whh, bih, bhh):
        wih_p = (R * wih).astype(np.float16)
        whh_p = (R * whh).astype(np.float16)
        b_p = (R[:, 0] * (bih + bhh)).astype(np.float32)
        return wih_p.T.copy(), whh_p.T.copy(), b_p  # lhsT layouts (128, 512)

    wih0_t, whh0_t, b0 = lstm_w(inp["wih0"], inp["whh0"], inp["bih0"], inp["bhh0"])
    wih1_t, whh1_t, b1 = lstm_w(inp["wih1"], inp["whh1"], inp["bih1"], inp["bhh1"])

    # bias_pack[k, l*256 + blk*128 + m]: blk0 rows (i,f); blk1 rows (2g, o)
    bias_pack = np.zeros((2, 512), np.float16)
    for l, b in enumerate((b0, b1)):
        bias_pack[0, l * 256:l * 256 + 128] = b[0:128]            # i
        bias_pack[1, l * 256:l * 256 + 128] = b[128:256]          # f
        bias_pack[0, l * 256 + 128:l * 256 + 256] = b[256:384]    # g (doubled)
        bias_pack[1, l * 256 + 128:l * 256 + 256] = b[384:512]    # o
    ones_pack = np.zeros((2, 512), np.float16)
    ones_pack[0, 0:256] = 1.0
    ones_pack[1, 256:512] = 1.0

    shared = {
        "fcw_t": fcw.T.copy().astype(np.float16),
        "fcb": fcb.reshape(H, 1).astype(np.float32),
        "wih0_t": wih0_t, "whh0_t": whh0_t,
        "wih1_t": wih1_t, "whh1_t": whh1_t,
        "bias_pack": bias_pack, "ones_pack": ones_pack,
    }
    x = inp["x"].reshape(N_CORES, B_SHARD, F_IN, T)
    maps = []
    for i in range(N_CORES):
        xa = np.ascontiguousarray(
            x[i, :, 0:128, :].transpose(1, 0, 2).astype(np.float16)).reshape(
            128, B_SHARD * T)
        # xb[P, p*64+f, b'*T+t] = x[32P + p*16 + b', 128+f, t]  (rows 52:64 pad)
        xbt = (x[i, :, 128:F_IN, :]
               .reshape(B_SHARD // (2 * FC_NB), 2, FC_NB, F_IN - 128, T)
               .transpose(0, 1, 3, 2, 4))  # (P, 2, 52, 16, 120)
        xb = np.zeros((B_SHARD // (2 * FC_NB), 64 + F_IN - 128, FC_NB * T),
                      np.float16)
        xb[:, 0:F_IN - 128] = xbt[:, 0].reshape(-1, F_IN - 128, FC_NB * T)
        xb[:, 64:64 + F_IN - 128] = xbt[:, 1].reshape(-1, F_IN - 128, FC_NB * T)
        maps.append(dict(shared, xa=xa, xb=xb))
    return maps


_NC_CACHE = []


def _get_nc():
    if not _NC_CACHE:
        _NC_CACHE.append(build_kernel())
    return _NC_CACHE[0]


def run_on_cores(inputs, **kw):
    nc = _get_nc()
    in_maps = prep_inputs(inputs)
    return run_bass_kernel_spmd(nc, in_maps, core_ids=list(range(N_CORES)), **kw)


def kernel(**inputs) -> np.ndarray:
    res = run_on_cores(inputs)
    outs = [res.results[i]["out"].T for i in range(N_CORES)]  # (256, 128) each
    return np.ascontiguousarray(np.concatenate(outs, axis=0)).astype(np.float32)


# revision 43
# speedup vs baseline: 3350.4505x; 3350.4505x over previous
"""Trainium2 Bass kernel for nn_Encoder (FC+BN+LeakyReLU -> 2-layer LSTM -> last h).

Data-parallel over 8 NeuronCores: each core handles B_SHARD=256 of the 2048
batch rows; small FC/BN/LSTM params are replicated.

Per-core pipeline (all shapes [partition, free], compute in fp16 with fp32
PSUM accumulation; measured end-to-end rel err ~1e-3):
  Stage A (FC):  host pre-transposes x to feature-major fp16 (xa: rows 0:128,
                 xb: the 52 tail rows of two 16-batch groups packed at
                 partitions 0 and 64 so every DMA is wide).  K-split matmuls
                 (128 + 52) accumulate fc_w'@x into PSUM; one Lrelu ACT op
                 applies the host-folded BN scale/bias + LeakyReLU and writes
                 h_fc (128, 256*120) fp16, resident in SBUF.
  Stage B (LSTM): 2 layers, 120 steps, software-pipelined (layer 1 lags LAG
                 steps so its work fills layer 0's dependency-chain bubbles).
                 Per step+layer, gates in order (i|f|g'|o) live in one
                 (128, 1024) fp32 PSUM tile (2 banks); g rows of the weights
                 and bias are pre-doubled on host so tanh comes out of the
                 sigmoid op: tanh(z) = 2*sigmoid(2z) - 1.
                   - biases via two K=2 matmuls against a constant 0/1 rhs
                   - 4 x-part matmuls (K=128, N=256) from h_fc / h0
                   - 4 h-part matmuls (K=128, N=256) from the recurrent state
                 Elementwise chain per step (sigmoid/tanh share one ACT
                 table set; TT = fp16 tensor_tensor at 2x mode):
                   s    = sigmoid(psum[:, :768])   (ACT)   s = [si, sf, sg']
                   so   = sigmoid(psum[:, 768:])   (ACT)
                   t2   = sf * c                   (DVE TT)
                   t1   = (sg' - 0.5) * si         (DVE STT) = i*g/2
                   c    = 2*t1 + t2                (DVE STT)
                   th   = tanh(c)                  (ACT)
                   h    = th * so                  (DVE TT)
  Output: h1 at t=119 DMA'd out as (128, 256) fp16 per core; host
          transposes/concats/casts to (2048, 128) fp32.
"""

from contextlib import ExitStack

import numpy as np

import concourse.bass as bass
import concourse.mybir as mybir
import concourse.tile as tile
from concourse import bacc
from concourse.bass_utils import run_bass_kernel_spmd

H = 128
T = 120
F_IN = 180
N_CORES = 8
B_SHARD = 256
BN_EPS = 1e-5
SLOPE = 0.01

FP32 = mybir.dt.float32
FP16 = mybir.dt.float16
AF = mybir.ActivationFunctionType
ALU = mybir.AluOpType

FC_NB = 16          # batches per x DMA group
FC_MM_NB = 4        # batches per FC matmul (N = 4*120 = 480)


def build_kernel(mode="full"):
    nc = bacc.Bacc("TRN2", target_bir_lowering=False, debug=False,
                   num_devices=N_CORES)

    xa_d = nc.dram_tensor("xa", (128, B_SHARD * T), FP16, kind="ExternalInput")
    xb_d = nc.dram_tensor("xb", (B_SHARD // (2 * FC_NB), 64 + F_IN - 128, FC_NB * T),
                          FP16, kind="ExternalInput")
    fcw_d = nc.dram_tensor("fcw_t", (F_IN, H), FP16, kind="ExternalInput")
    fcb_d = nc.dram_tensor("fcb", (H, 1), FP32, kind="ExternalInput")
    wih_d = [nc.dram_tensor(f"wih{l}_t", (H, 4 * H), FP16, kind="ExternalInput")
             for l in range(2)]
    whh_d = [nc.dram_tensor(f"whh{l}_t", (H, 4 * H), FP16, kind="ExternalInput")
             for l in range(2)]
    bias_d = nc.dram_tensor("bias_pack", (2, 512), FP16, kind="ExternalInput")
    ones_d = nc.dram_tensor("ones_pack", (2, 512), FP16, kind="ExternalInput")

    out_d = nc.dram_tensor("out", (H, B_SHARD), FP16, kind="ExternalOutput")

    ctx = ExitStack()
    with ctx:
        tc = ctx.enter_context(tile.TileContext(nc))
        consts = ctx.enter_context(tc.tile_pool(name="consts", bufs=1))
        state = ctx.enter_context(tc.tile_pool(name="state", bufs=1))

        # ---- constants into SBUF ----
        fcw_a = consts.tile([128, H], FP16, tag="fcw_a")
        # two copies of the 52-row tail weight, at base partitions 0 and 64,
        # matching the packed xb layout (matmul needs equal base partitions)
        fcw_b2 = consts.tile([64 + F_IN - 128, H], FP16, tag="fcw_b")
        nc.sync.dma_start(out=fcw_a, in_=fcw_d.ap()[0:128, :])
        nc.sync.dma_start(out=fcw_b2[0:F_IN - 128], in_=fcw_d.ap()[128:F_IN, :])
        nc.sync.dma_start(out=fcw_b2[64:64 + F_IN - 128], in_=fcw_d.ap()[128:F_IN, :])
        fcb = consts.tile([H, 1], FP32, tag="fcb")
        nc.sync.dma_start(out=fcb, in_=fcb_d.ap())
        wihT = []
        whhT = []
        for l in range(2):
            wt = consts.tile([H, 4 * H], FP16, tag=f"wih{l}")
            nc.sync.dma_start(out=wt, in_=wih_d[l].ap())
            wihT.append(wt)
            ht = consts.tile([H, 4 * H], FP16, tag=f"whh{l}")
            nc.sync.dma_start(out=ht, in_=whh_d[l].ap())
            whhT.append(ht)
        bias_t = consts.tile([2, 512], FP16, tag="bias")
        nc.sync.dma_start(out=bias_t, in_=bias_d.ap())
        ones_t = consts.tile([2, 512], FP16, tag="ones")
        nc.sync.dma_start(out=ones_t, in_=ones_d.ap())


        # ---- persistent state ----
        h_fc = state.tile([128, B_SHARD, T], FP16, tag="h_fc")
        h0_all = state.tile([128, T + 1, B_SHARD], FP16, tag="h0_all")
        h1_ring = state.tile([128, 2, B_SHARD], FP16, tag="h1_ring")
        c2 = [state.tile([128, B_SHARD], FP16, tag=f"c2_{l}", name=f"c2_{l}")
              for l in range(2)]
        nc.vector.memset(h0_all[:, 0, :], 0.0)
        nc.vector.memset(h1_ring[:, 0, :], 0.0)
        nc.vector.memset(c2[0], 0.0)
        nc.vector.memset(c2[1], 0.0)

        # ---- Stage A: FC + bias + LeakyReLU ----
        if mode == "null":
            nc.sync.dma_start(out=out_d.ap(), in_=h1_ring[:, T % 2, :])
            mode = "skip"
        with tc.tile_pool(name="fc_x", bufs=3) as xpool, \
             tc.tile_pool(name="fc_ps", bufs=4, space="PSUM") as fc_psum, \
             tc.tile_pool(name="fc_z", bufs=4) as zpool:
            FB = F_IN - 128  # 52 tail rows; two groups' tails share one DMA
            for grp in range(B_SHARD // FC_NB if mode != "skip" else 0):
                b0 = grp * FC_NB
                xa = xpool.tile([128, FC_NB * T], FP16, tag="xa")
                nc.sync.dma_start(
                    out=xa, in_=xa_d.ap()[:, b0 * T:(b0 + FC_NB) * T])
                if grp % 2 == 0:
                    xb = xpool.tile([64 + FB, FC_NB * T], FP16, tag="xb", name="xb")
                    nc.sync.dma_start(out=xb, in_=xb_d.ap()[grp // 2])
                    xb_lo, fcw_b = xb[0:FB], fcw_b2[0:FB]
                else:
                    xb_lo, fcw_b = xb[64:64 + FB], fcw_b2[64:64 + FB]
                for mm in range(FC_NB // FC_MM_NB):
                    s = mm * FC_MM_NB
                    ps = fc_psum.tile([128, FC_MM_NB * T], FP32, tag="ps")
                    rhs_a = xa[:, s * T:(s + FC_MM_NB) * T]
                    rhs_b = xb_lo[:, s * T:(s + FC_MM_NB) * T]
                    nc.tensor.matmul(ps, fcw_a, rhs_a, start=True, stop=False)
                    nc.tensor.matmul(ps, fcw_b, rhs_b, start=False, stop=True)
                    dst = h_fc[:, b0 + s:b0 + s + FC_MM_NB, :].rearrange(
                        "f b t -> f (b t)")
                    nc.scalar.activation(out=dst, in_=ps, func=AF.Lrelu,
                                         bias=fcb, scale=1.0, alpha=SLOPE)

            if mode == "skip":
                pass  # pools open/close empty
        if mode == "fc":
            nc.sync.dma_start(out=out_d.ap(), in_=h1_ring[:, T % 2, :])
            mode = "skip"
        # ---- Stage B: LSTM ----
        hfc_t = h_fc  # [128, b, t]; x-part rhs for layer 0 at step t: h_fc[:, :, t]
        LAG = 3       # layer-1 pipeline lag (ticks) so it fills layer-0 bubbles
        with tc.tile_pool(name="g_ps", bufs=2, space="PSUM") as gpsum, \
             tc.tile_pool(name="ifgo", bufs=3) as gpool, \
             tc.tile_pool(name="sc", bufs=4) as scpool:

            def lstm_step_a(l, t):
                """Matmuls + sigmoids + cell update; returns (th-input deps)."""
                if l == 0:
                    rhs_x = hfc_t[:, :, t]
                    rhs_h = h0_all[:, t, :]
                else:
                    rhs_x = h0_all[:, t + 1, :]
                    rhs_h = h1_ring[:, t % 2, :]
                ps = gpsum.tile([128, 1024], FP32, tag=f"g{l}", name=f"ps{l}")
                # biases via matmuls against 0/1 rhs: i/f (bank0, K=2, N=512),
                # g doubled + o (bank1, K=2, N=512)
                nc.tensor.matmul(ps[:, 0:512], bias_t[:, l * 256:l * 256 + 128],
                                 ones_t, start=True, stop=False)
                nc.tensor.matmul(ps[:, 512:1024],
                                 bias_t[:, l * 256 + 128:l * 256 + 256],
                                 ones_t, start=True, stop=False)
                for g in range(4):
                    nc.tensor.matmul(ps[:, g * 256:(g + 1) * 256],
                                     wihT[l][:, g * 128:(g + 1) * 128], rhs_x,
                                     start=False, stop=False)
                for g in range(4):
                    nc.tensor.matmul(ps[:, g * 256:(g + 1) * 256],
                                     whhT[l][:, g * 128:(g + 1) * 128], rhs_h,
                                     start=False, stop=(g >= 1))
                # one sigmoid over i, f, g', o (g rows pre-doubled:
                # tanh(z) = 2*sigmoid(2z) - 1)
                sifg = gpool.tile([128, 1024], FP16, tag=f"sifg{l}", name=f"sifg{l}")
                nc.scalar.activation(out=sifg[:, 0:768], in_=ps[:, 0:768],
                                     func=AF.Sigmoid)
                nc.scalar.activation(out=sifg[:, 768:1024], in_=ps[:, 768:1024],
                                     func=AF.Sigmoid)
                si = sifg[:, 0:256]
                sf = sifg[:, 256:512]
                sg = sifg[:, 512:768]
                so = sifg[:, 768:1024]
                t2 = scpool.tile([128, 256], FP16, tag=f"t2_{l}", name=f"t2_{l}")
                nc.vector.tensor_mul(t2, sf, c2[l])
                t1 = scpool.tile([128, 256], FP16, tag=f"t1_{l}", name=f"t1_{l}")
                nc.vector.scalar_tensor_tensor(out=t1, in0=sg, scalar=-0.5,
                                               in1=si, op0=ALU.add, op1=ALU.mult)
                nc.vector.scalar_tensor_tensor(out=c2[l], in0=t1, scalar=2.0,
                                               in1=t2, op0=ALU.mult, op1=ALU.add)
                return so

            def lstm_step_b(l, t, so):
                h_dst = (h0_all[:, t + 1, :] if l == 0
                         else h1_ring[:, (t + 1) % 2, :])
                th = scpool.tile([128, 256], FP16, tag=f"th{l}", name=f"th{l}")
                nc.scalar.activation(out=th, in_=c2[l], func=AF.Tanh)
                nc.vector.tensor_mul(h_dst, th, so)

            for tick in range(T + LAG if mode != "skip" else 0):
                so1 = lstm_step_a(1, tick - LAG) if tick >= LAG else None
                so0 = lstm_step_a(0, tick) if tick < T else None
                if so1 is not None:
                    lstm_step_b(1, tick - LAG, so1)
                if so0 is not None:
                    lstm_step_b(0, tick, so0)

            if mode != "skip":
                nc.sync.dma_start(out=out_d.ap(), in_=h1_ring[:, T % 2, :])

    nc.compile()
    return nc


def prep_inputs(inputs):
    """Host-side weight preprocessing; returns per-core in_maps."""
    inp = {k: np.asarray(v, dtype=np.float32) for k, v in inputs.items()}
    s = 1.0 / np.sqrt(inp["bn_var"] + BN_EPS)
    scale_p = s * inp["bn_g"]
    fcw = scale_p[:, None] * inp["fc_w"]          # (128, 180)
    fcb = (inp["fc_b"] - inp["bn_mean"]) * scale_p + inp["bn_b"]

    # Gate order on-chip matches PyTorch rows: i, f, g, o.  The g rows are
    # doubled so tanh(z) = 2*sigmoid(2z) - 1 comes out of the sigmoid op.
    R = np.ones((4 * H, 1), np.float32)
    R[2 * H:3 * H] = 2.0

    def lstm_w(wih, whh, bih, bhh):
        wih_p = (R * wih).astype(np.float16)
        whh_p = (R * whh).astype(np.float16)
        b_p = (R[:, 0] * (bih + bhh)).astype(np.float32)
        return wih_p.T.copy(), whh_p.T.copy(), b_p  # lhsT layouts (128, 512)

    wih0_t, whh0_t, b0 = lstm_w(inp["wih0"], inp["whh0"], inp["bih0"], inp["bhh0"])
    wih1_t, whh1_t, b1 = lstm_w(inp["wih1"], inp["whh1"], inp["bih1"], inp["bhh1"])

    # bias_pack[k, l*256 + blk*128 + m]: blk0 rows (i,f); blk1 rows (2g, o)
    bias_pack = np.zeros((2, 512), np.float16)
    for l, b in enumerate((b0, b1)):
        bias_pack[0, l * 256:l * 256 + 128] = b[0:128]            # i
        bias_pack[1, l * 256:l * 256 + 128] = b[128:256]          # f
        bias_pack[0, l * 256 + 128:l * 256 + 256] = b[256:384]    # g (doubled)
        bias_pack[1, l * 256 + 128:l * 256 + 256] = b[384:512]    # o
    ones_pack = np.zeros((2, 512), np.float16)
    ones_pack[0, 0:256] = 1.0
    ones_pack[1, 256:512] = 1.0

    shared = {
        "fcw_t": fcw.T.copy().astype(np.float16),
        "fcb": fcb.reshape(H, 1).astype(np.float32),
        "wih0_t": wih0_t, "whh0_t": whh0_t,
        "wih1_t": wih1_t, "whh1_t": whh1_t,
        "bias_pack": bias_pack, "ones_pack": ones_pack,
    }
    x = inp["x"].reshape(N_CORES, B_SHARD, F_IN, T)
    maps = []
    for i in range(N_CORES):
        xa = np.ascontiguousarray(
            x[i, :, 0:128, :].transpose(1, 0, 2).astype(np.float16)).reshape(
            128, B_SHARD * T)
        # xb[P, p*64+f, b'*T+t] = x[32P + p*16 + b', 128+f, t]  (rows 52:64 pad)
        xbt = (x[i, :, 128:F_IN, :]
               .reshape(B_SHARD // (2 * FC_NB), 2, FC_NB, F_IN - 128, T)
               .transpose(0, 1, 3, 2, 4))  # (P, 2, 52, 16, 120)
        xb = np.zeros((B_SHARD // (2 * FC_NB), 64 + F_IN - 128, FC_NB * T),
                      np.float16)
        xb[:, 0:F_IN - 128] = xbt[:, 0].reshape(-1, F_IN - 128, FC_NB * T)
        xb[:, 64:64 + F_IN - 128] = xbt[:, 1].reshape(-1, F_IN - 128, FC_NB * T)
        maps.append(dict(shared, xa=xa, xb=xb))
    return maps


_NC_CACHE = []


def _get_nc():
    if not _NC_CACHE:
        _NC_CACHE.append(build_kernel())
    return _NC_CACHE[0]


def run_on_cores(inputs, **kw):
    nc = _get_nc()
    in_maps = prep_inputs(inputs)
    return run_bass_kernel_spmd(nc, in_maps, core_ids=list(range(N_CORES)), **kw)


def kernel(**inputs) -> np.ndarray:
    res = run_on_cores(inputs)
    outs = [res.results[i]["out"].T for i in range(N_CORES)]  # (256, 128) each
    return np.ascontiguousarray(np.concatenate(outs, axis=0)).astype(np.float32)
